# revision 15
# baseline (speedup 1.0000x reference)
"""Trainium2 Bass kernel for ANE-Gemma MQA single-token decode attention.

Distribution over 8 NeuronCores:
  - QKV projection: output-row sharded (320 rows/core) + AllGather.
  - Attention: KV-cache sequence-sharded; per-core partial softcapped
    attention with fixed exp(s-50) stabilizer; ReduceScatter(add) gives
    core c the summed (acc, l) for head c.
  - O-projection: head-column sharded; per-core 2048-float partials are
    summed on the host.

Host-side prep is layout only: slicing, transposes, replication of tiny
constants, and reading the mask to select valid cache rows (exp(mask) is
folded into the shipped V rows / softmax-denominator column, which is
mathematically identical to the reference's additive mask).
"""

import numpy as np

N_CORES = 8
H = 8            # query heads
D = 256          # head dim
HID = 2048       # hidden
QKV_ROWS = (H + 2) * D          # 2560
ROWS_PER_CORE = QKV_ROWS // N_CORES  # 320
LAYER_INDEX = 5
SOFTCAP = 50.0
OWNER = N_CORES - 1  # core that contributes the freshly-written kv position

_GRAPH_CACHE = {}


def _split_excess_waits(nc):
    """Walrus in this environment accepts at most 1 semaphore wait per
    instruction (2 for EventSemaphore). Tile's wait assigner can emit more;
    hoist the excess into standalone EventSemaphore waits just before the
    instruction on the same engine stream."""
    import concourse.mybir as mybir

    uid = [0]
    for fn in nc.m.functions:
        for blk in fn.blocks:
            out = []
            for inst in blk.instructions:
                si = inst.sync_info
                cap = 2 if isinstance(inst, mybir.InstEventSemaphore) else 1
                if si is not None and si.on_wait and len(si.on_wait) > cap:
                    waits = list(si.on_wait)
                    keep, hoist = waits[-cap:], waits[:-cap]
                    while hoist:
                        chunk, hoist = hoist[:2], hoist[2:]
                        uid[0] += 1
                        out.append(mybir.InstEventSemaphore(
                            name=f"splitw-{uid[0]}",
                            ins=[], outs=[],
                            engine=inst.engine,
                            sync_info=mybir.SyncInfo(on_wait=chunk, on_update=[]),
                        ))
                    inst.sync_info = mybir.SyncInfo(
                        on_wait=keep, on_update=si.on_update)
                out.append(inst)
            if len(out) != len(blk.instructions):
                blk.instructions[:] = out
    return nc


def _build_graph(n_c, s_p, split_waits=True):
    """SPMD Bass graph. n_c real cache rows per core (multiple of 128); the
    new-kv vector occupies row n_c (partition 0 of the last seq tile);
    s_p = n_c + 128."""
    import concourse.bass as bass
    import concourse.mybir as mybir
    from concourse import masks, tile

    fp = mybir.dt.float32
    bf = mybir.dt.bfloat16
    AF = mybir.ActivationFunctionType
    nt = s_p // 128
    assert s_p == n_c + 128 and n_c % 128 == 0

    nc = bass.Bass(num_devices=N_CORES)

    # --- kernel I/O (per-core shards supplied by the host) ---
    # wqkvT carries the hidden-state vector as its last column (321 = 320+1)
    # so each qkv matmul depends on exactly one DMA.
    wq_p = nc.declare_dram_parameter(
        "wqkvT", [HID, ROWS_PER_CORE + 1], bf, isOutput=False)
    kt_p = nc.declare_dram_parameter("kT", [D, s_p], bf, isOutput=False)
    v_p = nc.declare_dram_parameter("vaug", [s_p, D + 1], bf, isOutput=False)
    ow_p = nc.declare_dram_parameter("owT", [D, HID], bf, isOutput=False)
    cst_p = nc.declare_dram_parameter("consts", [36, D], fp, isOutput=False)
    out_p = nc.declare_dram_parameter("out", [1, HID], fp, isOutput=True)

    # --- internal DRAM bounce buffers for collectives ---
    cc0_in = nc.dram_tensor("cc0_in", [1, 8], fp)
    cc0_out = nc.dram_tensor("cc0_out", [8, 8], fp, addr_space="Shared")
    cc1_in = nc.dram_tensor("cc1_in", [1, ROWS_PER_CORE], fp)
    cc1_out = nc.dram_tensor("cc1_out", [H + 2, D], fp, addr_space="Shared")
    cc2_in = nc.dram_tensor("cc2_in", [H, D + 1], fp)
    cc2_out = nc.dram_tensor("cc2_out", [1, D + 1], fp)
    rgroups = [list(range(N_CORES))]

    with tile.TileContext(nc) as tc:
        with (
            tc.tile_pool(name="wp", bufs=1) as wp,
            tc.tile_pool(name="sp", bufs=1) as sp,
            tc.tile_pool(name="pp", bufs=8, space="PSUM") as pp,
        ):
            # warm up the collectives path: the first collective of an
            # execution pays ~25-30us of setup; do it on a dummy buffer
            # concurrently with the input DMA phase.
            nc.gpsimd.collective_compute(
                "AllGather", mybir.AluOpType.bypass, replica_groups=rgroups,
                ins=[cc0_in[:]], outs=[cc0_out[:]],
            )

            # ---------------- DMA in ----------------
            # critical path first (sync queue): qkv weight slices (+h), consts
            wqv = wq_p.rearrange("(a p) r -> a p r", p=128)  # [16,128,321]
            wq = []
            for a in range(4):
                t = wp.tile([128, 4, ROWS_PER_CORE + 1], bf,
                            name=f"wq{a}", tag=f"wq{a}")
                nc.sync.dma_start(
                    out=t[:],
                    in_=wqv[4 * a:4 * (a + 1)].rearrange("a p r -> p a r"),
                )
                wq.append(t)
            csb = wp.tile([9, 4, D], fp)
            nc.sync.dma_start(
                out=csb[:], in_=cst_p.rearrange("(j r) d -> r j d", r=9))
            cw = csb[:, 0, :]      # norm weights: q rows raw, k row 15+16*kw
            ccos = csb[:, 1, :]
            csin = csb[:, 2, :]
            cfac = csb[0:1, 3, 0:1]  # new-kv mask factor
            # bulk loads on the scalar HWDGE queue: K^T, V, o_w^T
            kt0 = wp.tile([128, s_p], bf)
            kt1 = wp.tile([128, s_p], bf)
            nc.scalar.dma_start(out=kt0[:], in_=kt_p[0:128, :])
            nc.scalar.dma_start(out=kt1[:], in_=kt_p[128:256, :])
            vt = []
            for t_i in range(nt):
                t = wp.tile([128, D + 1], bf, name=f"vt{t_i}", tag=f"vt{t_i}")
                nc.scalar.dma_start(
                    out=t[:], in_=v_p[128 * t_i:128 * (t_i + 1), :]
                )
                vt.append(t)
            ow = []
            for j in range(2):
                for b in range(4):
                    t = wp.tile([128, 512], bf, name=f"ow{j}{b}", tag=f"ow{j}{b}")
                    nc.scalar.dma_start(
                        out=t[:],
                        in_=ow_p[128 * j:128 * (j + 1), 512 * b:512 * (b + 1)],
                    )
                    ow.append(t)

            id16 = wp.tile([16, 16], fp)
            masks.make_identity(nc, id16[:])

            # ---------------- QKV projection (partial rows) ----------------
            psq = pp.tile([1, ROWS_PER_CORE], fp, tag="ps")
            for k in range(16):
                a, j = k // 4, k % 4
                nc.tensor.matmul(
                    psq[:],
                    lhsT=wq[a][:, j, ROWS_PER_CORE:ROWS_PER_CORE + 1],
                    rhs=wq[a][:, j, 0:ROWS_PER_CORE],
                    start=(k == 0), stop=(k == 15),
                )
            qkvp = sp.tile([1, ROWS_PER_CORE], fp)
            nc.scalar.activation(qkvp[:], psq[:], AF.Copy)
            nc.gpsimd.dma_start(out=cc1_in[:], in_=qkvp[:])

            # ---------------- AllGather qkv ----------------
            nc.gpsimd.collective_compute(
                "AllGather", mybir.AluOpType.bypass, replica_groups=rgroups,
                ins=[cc1_in[:]], outs=[cc1_out[:]],
            )
            qkn = sp.tile([9, D], fp)      # q heads + k
            vrow = sp.tile([1, D], fp)     # raw v
            nc.gpsimd.dma_start(out=qkn[:], in_=cc1_out[0:9, :])
            nc.gpsimd.dma_start(out=vrow[:], in_=cc1_out[9:10, :])

            # ---------------- RMSNorm + RoPE (q heads + k) ----------------
            mv = sp.tile([9, 1], fp)
            nc.vector.tensor_reduce(
                mv[:], qkn[:], axis=mybir.AxisListType.X, op=mybir.AluOpType.max,
                apply_absolute_value=True,
            )
            mv2 = sp.tile([9, 1], fp)
            nc.vector.tensor_scalar_max(mv2[:], mv[:], 2.0 ** -24)
            rmv = sp.tile([9, 1], fp)
            nc.vector.reciprocal(rmv[:], mv2[:])
            xs = sp.tile([9, D], fp)
            nc.vector.tensor_scalar_mul(xs[:], qkn[:], rmv[:])
            xs2 = sp.tile([9, D], fp)
            ss = sp.tile([9, 1], fp)
            nc.scalar.activation(xs2[:], xs[:], AF.Square, accum_out=ss[:])
            sq = sp.tile([9, 1], fp)
            nc.scalar.activation(sq[:], ss[:], AF.Sqrt)
            rs = sp.tile([9, 1], fp)
            nc.vector.reciprocal(rs[:], sq[:])
            # effective per-row scale: q rows rs*sqrt(D)*SCALING = rs,
            # k row: rs (its *16 is baked into cw row 8 by the host)
            xn = sp.tile([9, D], fp)
            nc.vector.tensor_scalar_mul(xn[:], xs[:], rs[:])
            # * (1 + w)
            t1 = sp.tile([9, D], fp)
            nc.vector.tensor_mul(t1[:], xn[:], cw[:])
            xnw = sp.tile([9, D], fp)
            nc.vector.tensor_add(xnw[:], xn[:], t1[:])
            # rope
            rot = sp.tile([9, D], fp)
            nc.vector.tensor_scalar_mul(rot[:, 0:128], xnw[:, 128:256], -1.0)
            nc.vector.tensor_copy(rot[:, 128:256], xnw[:, 0:128])
            ca = sp.tile([9, D], fp)
            nc.vector.tensor_mul(ca[:], xnw[:], ccos[:])
            cb = sp.tile([9, D], fp)
            nc.vector.tensor_mul(cb[:], rot[:], csin[:])
            qr = sp.tile([9, D], fp)
            nc.vector.tensor_add(qr[:], ca[:], cb[:])
            # raw v scaled by the per-core new-kv factor (exp(mask[p]) or 0)
            vscl = sp.tile([1, D], fp)
            nc.vector.tensor_scalar_mul(vscl[:], vrow[:], cfac[:])

            # ---------------- transpose new q/k ----------------
            pst0 = pp.tile([128, 9], fp, tag="ps")
            pst1 = pp.tile([128, 9], fp, tag="ps")
            nc.tensor.transpose(pst0[:], qr[:, 0:128], id16[0:9, 0:9])
            nc.tensor.transpose(pst1[:], qr[:, 128:256], id16[0:9, 0:9])
            qt0 = sp.tile([128, H], bf)
            qt1 = sp.tile([128, H], bf)
            nc.vector.tensor_copy(qt0[:], pst0[:, 0:H])
            nc.vector.tensor_copy(qt1[:], pst1[:, 0:H])
            # append new k as column n_c of K^T
            nc.vector.tensor_copy(kt0[:, n_c:n_c + 1], pst0[:, H:H + 1])
            nc.vector.tensor_copy(kt1[:, n_c:n_c + 1], pst1[:, H:H + 1])
            # append new v as row n_c = partition 0 of the last V tile
            nc.vector.tensor_copy(vt[nt - 1][0:1, 0:D], vscl[:])

            # ---------------- scores + softcap softmax numerators ----------------
            pss = pp.tile([128, nt * H], fp, tag="ps")
            for t_i in range(nt):
                nc.tensor.matmul(
                    pss[:, H * t_i:H * (t_i + 1)],
                    lhsT=kt0[:, 128 * t_i:128 * (t_i + 1)], rhs=qt0[:],
                    start=True, stop=False,
                )
                nc.tensor.matmul(
                    pss[:, H * t_i:H * (t_i + 1)],
                    lhsT=kt1[:, 128 * t_i:128 * (t_i + 1)], rhs=qt1[:],
                    start=False, stop=True,
                )
            nb = sp.tile([128, 1], fp)
            nc.gpsimd.memset(nb[:], -SOFTCAP)
            t40 = sp.tile([128, nt * H], fp)
            nc.scalar.activation(t40[:], pss[:], AF.Tanh, scale=1.0 / SOFTCAP)
            u40 = sp.tile([128, nt * H], bf)
            nc.scalar.activation(u40[:], t40[:], AF.Exp, bias=nb[:], scale=SOFTCAP)

            # ---------------- probs @ [V | 1] ----------------
            psav = pp.tile([H, D + 1], fp, tag="ps")
            for t_i in range(nt):
                nc.tensor.matmul(
                    psav[:], lhsT=u40[:, H * t_i:H * (t_i + 1)], rhs=vt[t_i][:],
                    start=(t_i == 0), stop=(t_i == nt - 1),
                )
            avs = sp.tile([H, D + 1], fp)
            nc.vector.tensor_copy(avs[:], psav[:])
            nc.gpsimd.dma_start(out=cc2_in[:], in_=avs[:])

            # ---------------- ReduceScatter partial (acc, l) ----------------
            nc.gpsimd.collective_compute(
                "ReduceScatter", mybir.AluOpType.add, replica_groups=rgroups,
                ins=[cc2_in[:]], outs=[cc2_out[:]],
            )
            accflat = sp.tile([1, D + 1], fp)
            nc.gpsimd.dma_start(out=accflat[:], in_=cc2_out[:])
            rl = sp.tile([1, 1], fp)
            nc.vector.reciprocal(rl[:], accflat[0:1, D:D + 1])
            pta = pp.tile([128, 1], fp, tag="ps")
            ptb = pp.tile([128, 1], fp, tag="ps")
            nc.tensor.transpose(pta[:], accflat[0:1, 0:128], id16[0:1, 0:1])
            nc.tensor.transpose(ptb[:], accflat[0:1, 128:256], id16[0:1, 0:1])
            acc2 = sp.tile([128, 2], bf)
            nc.vector.tensor_copy(acc2[:, 0:1], pta[:])
            nc.vector.tensor_copy(acc2[:, 1:2], ptb[:])

            # ---------------- O-projection partial ----------------
            osb = sp.tile([1, HID], fp)
            for b in range(4):
                pso = pp.tile([1, 512], fp, name=f"pso{b}", tag="ps")
                nc.tensor.matmul(pso[:], lhsT=acc2[:, 0:1], rhs=ow[b][:],
                                 start=True, stop=False)
                nc.tensor.matmul(pso[:], lhsT=acc2[:, 1:2], rhs=ow[4 + b][:],
                                 start=False, stop=True)
                nc.vector.tensor_scalar_mul(
                    osb[0:1, 512 * b:512 * (b + 1)], pso[:], rl[:]
                )
            nc.gpsimd.dma_start(out=out_p[:], in_=osb[:])

    return _split_excess_waits(nc) if split_waits else nc


def _prep_shards(hidden_states, cos, sin, kv_write_indices, k_cache, v_cache,
                 mask, qkv_w, o_w, q_norm_w, k_norm_w):
    import ml_dtypes
    f32 = np.float32
    bf16 = ml_dtypes.bfloat16
    p = int(np.asarray(kv_write_indices))
    mask_flat = np.asarray(mask, f32).reshape(-1)
    seq = mask_flat.shape[0]

    valid = np.nonzero(mask_flat > -1e8)[0]
    rows = valid[valid != p]
    n_c = max(1, (len(rows) + N_CORES - 1) // N_CORES)
    n_c = ((n_c + 127) // 128) * 128   # new-kv row lands at partition 0
    s_p = n_c + 128

    idx = np.zeros(N_CORES * n_c, np.int64)
    idx[:len(rows)] = rows
    live = np.zeros(N_CORES * n_c, bool)
    live[:len(rows)] = True
    idx = idx.reshape(N_CORES, n_c)
    live = live.reshape(N_CORES, n_c)

    k_l = np.asarray(k_cache, f32)[LAYER_INDEX, 0]
    v_l = np.asarray(v_cache, f32)[LAYER_INDEX, 0]

    h_vec = np.asarray(hidden_states, f32).reshape(HID)
    wqT = np.asarray(qkv_w, f32).T  # [HID, 2560]
    cos_f = np.asarray(cos, f32).reshape(D)
    sin_f = np.asarray(sin, f32).reshape(D)
    qw = np.asarray(q_norm_w, f32).reshape(D)
    kw = np.asarray(k_norm_w, f32).reshape(D)

    in_maps = []
    for c in range(N_CORES):
        rows_c = idx[c]
        live_c = live[c]
        # mask factor per shipped row: exp(mask) for live rows, 0 for padding
        mfac = np.zeros(n_c, f32)
        mfac[live_c] = np.exp(
            mask_flat[rows_c[live_c]].astype(np.float64)).astype(f32)

        ktc = np.zeros((D, s_p), bf16)
        ktc[:, :n_c] = k_l[rows_c].T.astype(bf16)
        vc = np.zeros((s_p, D + 1), bf16)
        vc[:n_c, :D] = (v_l[rows_c] * mfac[:, None]).astype(bf16)
        vc[:n_c, D] = mfac.astype(bf16)
        # new-kv slot at row n_c: factor = exp(mask[p]) on the owner core only
        nf = f32(0.0)
        if c == OWNER and 0 <= p < seq:
            nf = np.exp(np.float64(mask_flat[p])).astype(f32)
        vc[n_c, D] = bf16(nf)

        consts = np.zeros((36, D), f32)
        consts[0:8] = qw
        consts[8] = 15.0 + 16.0 * kw   # (1+w') = 16*(1+kw): folds in sqrt(D)
        consts[9:18] = cos_f
        consts[18:27] = sin_f
        consts[27, 0] = nf

        wqc = np.zeros((HID, ROWS_PER_CORE + 1), bf16)
        wqc[:, :ROWS_PER_CORE] = wqT[
            :, ROWS_PER_CORE * c:ROWS_PER_CORE * (c + 1)].astype(bf16)
        wqc[:, ROWS_PER_CORE] = h_vec.astype(bf16)

        in_maps.append(dict(
            wqkvT=wqc,
            kT=ktc,
            vaug=vc,
            owT=np.ascontiguousarray(
                np.asarray(o_w, f32)[:, D * c:D * (c + 1)].T.astype(bf16)),
            consts=consts,
        ))
    return in_maps, n_c, s_p


def kernel(**inputs):
    from concourse.bass_utils import run_bass_kernel_spmd

    in_maps, n_c, s_p = _prep_shards(**inputs)
    key = (n_c, s_p)
    if key not in _GRAPH_CACHE:
        _GRAPH_CACHE[key] = _build_graph(n_c, s_p)
    nc = _GRAPH_CACHE[key]

    res = run_bass_kernel_spmd(nc, in_maps, core_ids=list(range(N_CORES)))
    out = np.zeros(HID, np.float64)
    for r in res.results:
        out += r["out"].reshape(HID).astype(np.float64)
    return out.astype(np.float32).reshape(1, HID, 1, 1)


# revision 16
# speedup vs baseline: 1.1291x; 1.1291x over previous
"""Trainium2 Bass kernel for ANE-Gemma MQA single-token decode attention.

Distribution over 8 NeuronCores:
  - QKV projection: output-row sharded (320 rows/core) + AllGather.
  - Attention: KV-cache sequence-sharded; per-core partial softcapped
    attention with fixed exp(s-50) stabilizer; ReduceScatter(add) gives
    core c the summed (acc, l) for head c.
  - O-projection: head-column sharded; per-core 2048-float partials are
    summed on the host.

Host-side prep is layout only: slicing, transposes, replication of tiny
constants, and reading the mask to select valid cache rows (exp(mask) is
folded into the shipped V rows / softmax-denominator column, which is
mathematically identical to the reference's additive mask).
"""

import numpy as np

N_CORES = 8
H = 8            # query heads
D = 256          # head dim
HID = 2048       # hidden
QKV_ROWS = (H + 2) * D          # 2560
ROWS_PER_CORE = QKV_ROWS // N_CORES  # 320
LAYER_INDEX = 5
SOFTCAP = 50.0
OWNER = N_CORES - 1  # core that contributes the freshly-written kv position

_GRAPH_CACHE = {}


def _split_excess_waits(nc):
    """Walrus in this environment accepts at most 1 semaphore wait per
    instruction (2 for EventSemaphore). Tile's wait assigner can emit more;
    hoist the excess into standalone EventSemaphore waits just before the
    instruction on the same engine stream."""
    import concourse.mybir as mybir

    uid = [0]
    for fn in nc.m.functions:
        for blk in fn.blocks:
            out = []
            for inst in blk.instructions:
                si = inst.sync_info
                cap = 2 if isinstance(inst, mybir.InstEventSemaphore) else 1
                if si is not None and si.on_wait and len(si.on_wait) > cap:
                    waits = list(si.on_wait)
                    keep, hoist = waits[-cap:], waits[:-cap]
                    while hoist:
                        chunk, hoist = hoist[:2], hoist[2:]
                        uid[0] += 1
                        out.append(mybir.InstEventSemaphore(
                            name=f"splitw-{uid[0]}",
                            ins=[], outs=[],
                            engine=inst.engine,
                            sync_info=mybir.SyncInfo(on_wait=chunk, on_update=[]),
                        ))
                    inst.sync_info = mybir.SyncInfo(
                        on_wait=keep, on_update=si.on_update)
                out.append(inst)
            if len(out) != len(blk.instructions):
                blk.instructions[:] = out
    return nc


def _build_graph(n_c, s_p, split_waits=True):
    """SPMD Bass graph. n_c real cache rows per core (multiple of 128); the
    new-kv vector occupies row n_c (partition 0 of the last seq tile);
    s_p = n_c + 128."""
    import concourse.bass as bass
    import concourse.mybir as mybir
    from concourse import masks, tile

    fp = mybir.dt.float32
    bf = mybir.dt.bfloat16
    AF = mybir.ActivationFunctionType
    nt = s_p // 128
    assert s_p == n_c + 128 and n_c % 128 == 0

    nc = bass.Bass(num_devices=N_CORES)

    # --- kernel I/O (per-core shards supplied by the host) ---
    # wqkvT carries the hidden-state vector as its last column (321 = 320+1)
    # so each qkv matmul depends on exactly one DMA.
    wq_p = nc.declare_dram_parameter(
        "wqkvT", [HID, ROWS_PER_CORE + 1], bf, isOutput=False)
    kt_p = nc.declare_dram_parameter("kT", [D, s_p], bf, isOutput=False)
    v_p = nc.declare_dram_parameter("vaug", [s_p, D + 1], bf, isOutput=False)
    ow_p = nc.declare_dram_parameter("owT", [D, HID], bf, isOutput=False)
    cst_p = nc.declare_dram_parameter("consts", [36, D], fp, isOutput=False)
    out_p = nc.declare_dram_parameter("out", [1, HID], fp, isOutput=True)

    # --- internal DRAM bounce buffers for collectives ---
    cc1_in = nc.dram_tensor("cc1_in", [1, ROWS_PER_CORE], fp)
    cc1_out = nc.dram_tensor("cc1_out", [H + 2, D], fp, addr_space="Shared")
    cc2_in = nc.dram_tensor("cc2_in", [H, D + 1], fp)
    cc2_out = nc.dram_tensor("cc2_out", [1, D + 1], fp)
    rgroups = [list(range(N_CORES))]

    with tile.TileContext(nc) as tc:
        with (
            tc.tile_pool(name="wp", bufs=1) as wp,
            tc.tile_pool(name="sp", bufs=1) as sp,
            tc.tile_pool(name="pp", bufs=8, space="PSUM") as pp,
        ):
            # ---------------- DMA in ----------------
            # critical path first (sync queue): qkv weight slices (+h), consts
            wqv = wq_p.rearrange("(a p) r -> a p r", p=128)  # [16,128,321]
            wq = []
            for a in range(4):
                t = wp.tile([128, 4, ROWS_PER_CORE + 1], bf,
                            name=f"wq{a}", tag=f"wq{a}")
                nc.sync.dma_start(
                    out=t[:],
                    in_=wqv[4 * a:4 * (a + 1)].rearrange("a p r -> p a r"),
                )
                wq.append(t)
            csb = wp.tile([9, 4, D], fp)
            nc.sync.dma_start(
                out=csb[:], in_=cst_p.rearrange("(j r) d -> r j d", r=9))
            cw = csb[:, 0, :]      # norm weights: q rows raw, k row 15+16*kw
            ccos = csb[:, 1, :]
            csin = csb[:, 2, :]
            cfac = csb[0:1, 3, 0:1]  # new-kv mask factor
            # bulk loads on the scalar HWDGE queue: K^T, V, o_w^T
            kt0 = wp.tile([128, s_p], bf)
            kt1 = wp.tile([128, s_p], bf)
            nc.scalar.dma_start(out=kt0[:], in_=kt_p[0:128, :])
            nc.scalar.dma_start(out=kt1[:], in_=kt_p[128:256, :])
            vt = []
            for t_i in range(nt):
                t = wp.tile([128, D + 1], bf, name=f"vt{t_i}", tag=f"vt{t_i}")
                nc.scalar.dma_start(
                    out=t[:], in_=v_p[128 * t_i:128 * (t_i + 1), :]
                )
                vt.append(t)
            ow = []
            for j in range(2):
                for b in range(4):
                    t = wp.tile([128, 512], bf, name=f"ow{j}{b}", tag=f"ow{j}{b}")
                    nc.scalar.dma_start(
                        out=t[:],
                        in_=ow_p[128 * j:128 * (j + 1), 512 * b:512 * (b + 1)],
                    )
                    ow.append(t)

            id16 = wp.tile([16, 16], fp)
            masks.make_identity(nc, id16[:])

            # ---------------- QKV projection (partial rows) ----------------
            psq = pp.tile([1, ROWS_PER_CORE], fp, tag="ps")
            for k in range(16):
                a, j = k // 4, k % 4
                nc.tensor.matmul(
                    psq[:],
                    lhsT=wq[a][:, j, ROWS_PER_CORE:ROWS_PER_CORE + 1],
                    rhs=wq[a][:, j, 0:ROWS_PER_CORE],
                    start=(k == 0), stop=(k == 15),
                )
            qkvp = sp.tile([1, ROWS_PER_CORE], fp)
            nc.scalar.activation(qkvp[:], psq[:], AF.Copy)
            nc.gpsimd.dma_start(out=cc1_in[:], in_=qkvp[:])

            # ---------------- AllGather qkv ----------------
            nc.gpsimd.collective_compute(
                "AllGather", mybir.AluOpType.bypass, replica_groups=rgroups,
                ins=[cc1_in[:]], outs=[cc1_out[:]],
            )
            qkn = sp.tile([9, D], fp)      # q heads + k
            vrow = sp.tile([1, D], fp)     # raw v
            nc.sync.dma_start(out=qkn[:], in_=cc1_out[0:9, :])
            nc.scalar.dma_start(out=vrow[:], in_=cc1_out[9:10, :])

            # ---------------- RMSNorm + RoPE (q heads + k) ----------------
            # x/||x||*sqrt(D) == ane_rmsnorm's max-prenormalized form in
            # exact arithmetic; f32 cannot overflow at these magnitudes.
            xs2 = sp.tile([9, D], fp)
            nc.vector.tensor_mul(xs2[:], qkn[:], qkn[:])
            ss = sp.tile([9, 1], fp)
            nc.vector.tensor_reduce(
                ss[:], xs2[:], axis=mybir.AxisListType.X, op=mybir.AluOpType.add)
            sq = sp.tile([9, 1], fp)
            nc.scalar.activation(sq[:], ss[:], AF.Sqrt)
            rs = sp.tile([9, 1], fp)
            nc.vector.reciprocal(rs[:], sq[:])
            # q rows: rs*sqrt(D)*SCALING = rs; k row's *16 and the (1+w)
            # offset are baked into cw by the host (cw = 1+w, k: 16*(1+kw)).
            xn = sp.tile([9, D], fp)
            nc.vector.tensor_scalar_mul(xn[:], qkn[:], rs[:])
            xnw = sp.tile([9, D], fp)
            nc.vector.tensor_mul(xnw[:], xn[:], cw[:])
            # rope, exploiting cos/sin half-duplication (emb = [freqs, freqs])
            ca = sp.tile([9, D], fp)
            nc.vector.tensor_mul(ca[:], xnw[:], ccos[:])
            cb = sp.tile([9, D], fp)
            nc.vector.tensor_mul(cb[:], xnw[:], csin[:])
            qr = sp.tile([9, D], fp)
            nc.vector.tensor_sub(qr[:, 0:128], ca[:, 0:128], cb[:, 128:256])
            nc.vector.tensor_add(qr[:, 128:256], ca[:, 128:256], cb[:, 0:128])
            # raw v scaled by the per-core new-kv factor (exp(mask[p]) or 0)
            vscl = sp.tile([1, D], fp)
            nc.vector.tensor_scalar_mul(vscl[:], vrow[:], cfac[:])

            # ---------------- transpose new q/k ----------------
            pst0 = pp.tile([128, 9], fp, tag="ps")
            pst1 = pp.tile([128, 9], fp, tag="ps")
            nc.tensor.transpose(pst0[:], qr[:, 0:128], id16[0:9, 0:9])
            nc.tensor.transpose(pst1[:], qr[:, 128:256], id16[0:9, 0:9])
            qt0 = sp.tile([128, H], bf)
            qt1 = sp.tile([128, H], bf)
            nc.vector.tensor_copy(qt0[:], pst0[:, 0:H])
            nc.vector.tensor_copy(qt1[:], pst1[:, 0:H])
            # append new k as column n_c of K^T
            nc.vector.tensor_copy(kt0[:, n_c:n_c + 1], pst0[:, H:H + 1])
            nc.vector.tensor_copy(kt1[:, n_c:n_c + 1], pst1[:, H:H + 1])
            # append new v as row n_c = partition 0 of the last V tile
            nc.vector.tensor_copy(vt[nt - 1][0:1, 0:D], vscl[:])

            # ---------------- scores + softcap softmax numerators ----------------
            pss = pp.tile([128, nt * H], fp, tag="ps")
            for t_i in range(nt):
                nc.tensor.matmul(
                    pss[:, H * t_i:H * (t_i + 1)],
                    lhsT=kt0[:, 128 * t_i:128 * (t_i + 1)], rhs=qt0[:],
                    start=True, stop=False,
                )
                nc.tensor.matmul(
                    pss[:, H * t_i:H * (t_i + 1)],
                    lhsT=kt1[:, 128 * t_i:128 * (t_i + 1)], rhs=qt1[:],
                    start=False, stop=True,
                )
            nb = sp.tile([128, 1], fp)
            nc.gpsimd.memset(nb[:], -SOFTCAP)
            t40 = sp.tile([128, nt * H], fp)
            nc.scalar.activation(t40[:], pss[:], AF.Tanh, scale=1.0 / SOFTCAP)
            u40 = sp.tile([128, nt * H], bf)
            nc.scalar.activation(u40[:], t40[:], AF.Exp, bias=nb[:], scale=SOFTCAP)

            # ---------------- probs @ [V | 1] ----------------
            psav = pp.tile([H, D + 1], fp, tag="ps")
            for t_i in range(nt):
                nc.tensor.matmul(
                    psav[:], lhsT=u40[:, H * t_i:H * (t_i + 1)], rhs=vt[t_i][:],
                    start=(t_i == 0), stop=(t_i == nt - 1),
                )
            avs = sp.tile([H, D + 1], fp)
            nc.vector.tensor_copy(avs[:], psav[:])
            nc.sync.dma_start(out=cc2_in[:], in_=avs[:])

            # ---------------- ReduceScatter partial (acc, l) ----------------
            nc.gpsimd.collective_compute(
                "ReduceScatter", mybir.AluOpType.add, replica_groups=rgroups,
                ins=[cc2_in[:]], outs=[cc2_out[:]],
            )
            accflat = sp.tile([1, D + 1], fp)
            nc.sync.dma_start(out=accflat[:], in_=cc2_out[:])
            rl = sp.tile([1, 1], fp)
            nc.vector.reciprocal(rl[:], accflat[0:1, D:D + 1])
            pta = pp.tile([128, 1], fp, tag="ps")
            ptb = pp.tile([128, 1], fp, tag="ps")
            nc.tensor.transpose(pta[:], accflat[0:1, 0:128], id16[0:1, 0:1])
            nc.tensor.transpose(ptb[:], accflat[0:1, 128:256], id16[0:1, 0:1])
            acc2 = sp.tile([128, 2], bf)
            nc.vector.tensor_copy(acc2[:, 0:1], pta[:])
            nc.vector.tensor_copy(acc2[:, 1:2], ptb[:])

            # ---------------- O-projection partial ----------------
            osb = sp.tile([1, HID], fp)
            for b in range(4):
                pso = pp.tile([1, 512], fp, name=f"pso{b}", tag="ps")
                nc.tensor.matmul(pso[:], lhsT=acc2[:, 0:1], rhs=ow[b][:],
                                 start=True, stop=False)
                nc.tensor.matmul(pso[:], lhsT=acc2[:, 1:2], rhs=ow[4 + b][:],
                                 start=False, stop=True)
                if b % 2 == 0:
                    nc.vector.tensor_scalar_mul(
                        osb[0:1, 512 * b:512 * (b + 1)], pso[:], rl[:])
                else:
                    nc.scalar.activation(
                        osb[0:1, 512 * b:512 * (b + 1)], pso[:], AF.Copy,
                        scale=rl[:])
            nc.sync.dma_start(out=out_p[:], in_=osb[:])

    return _split_excess_waits(nc) if split_waits else nc


def _prep_shards(hidden_states, cos, sin, kv_write_indices, k_cache, v_cache,
                 mask, qkv_w, o_w, q_norm_w, k_norm_w):
    import ml_dtypes
    f32 = np.float32
    bf16 = ml_dtypes.bfloat16
    p = int(np.asarray(kv_write_indices))
    mask_flat = np.asarray(mask, f32).reshape(-1)
    seq = mask_flat.shape[0]

    valid = np.nonzero(mask_flat > -1e8)[0]
    rows = valid[valid != p]
    n_c = max(1, (len(rows) + N_CORES - 1) // N_CORES)
    n_c = ((n_c + 127) // 128) * 128   # new-kv row lands at partition 0
    s_p = n_c + 128

    idx = np.zeros(N_CORES * n_c, np.int64)
    idx[:len(rows)] = rows
    live = np.zeros(N_CORES * n_c, bool)
    live[:len(rows)] = True
    idx = idx.reshape(N_CORES, n_c)
    live = live.reshape(N_CORES, n_c)

    k_l = np.asarray(k_cache, f32)[LAYER_INDEX, 0]
    v_l = np.asarray(v_cache, f32)[LAYER_INDEX, 0]

    h_vec = np.asarray(hidden_states, f32).reshape(HID)
    wqT = np.asarray(qkv_w, f32).T  # [HID, 2560]
    cos_f = np.asarray(cos, f32).reshape(D)
    sin_f = np.asarray(sin, f32).reshape(D)
    qw = np.asarray(q_norm_w, f32).reshape(D)
    kw = np.asarray(k_norm_w, f32).reshape(D)

    in_maps = []
    for c in range(N_CORES):
        rows_c = idx[c]
        live_c = live[c]
        # mask factor per shipped row: exp(mask) for live rows, 0 for padding
        mfac = np.zeros(n_c, f32)
        mfac[live_c] = np.exp(
            mask_flat[rows_c[live_c]].astype(np.float64)).astype(f32)

        ktc = np.zeros((D, s_p), bf16)
        ktc[:, :n_c] = k_l[rows_c].T.astype(bf16)
        vc = np.zeros((s_p, D + 1), bf16)
        vc[:n_c, :D] = (v_l[rows_c] * mfac[:, None]).astype(bf16)
        vc[:n_c, D] = mfac.astype(bf16)
        # new-kv slot at row n_c: factor = exp(mask[p]) on the owner core only
        nf = f32(0.0)
        if c == OWNER and 0 <= p < seq:
            nf = np.exp(np.float64(mask_flat[p])).astype(f32)
        vc[n_c, D] = bf16(nf)

        consts = np.zeros((36, D), f32)
        consts[0:8] = 1.0 + qw
        consts[8] = 16.0 + 16.0 * kw   # 16*(1+kw): folds in sqrt(D)
        consts[9:18] = cos_f
        consts[18:27] = sin_f
        consts[27, 0] = nf

        wqc = np.zeros((HID, ROWS_PER_CORE + 1), bf16)
        wqc[:, :ROWS_PER_CORE] = wqT[
            :, ROWS_PER_CORE * c:ROWS_PER_CORE * (c + 1)].astype(bf16)
        wqc[:, ROWS_PER_CORE] = h_vec.astype(bf16)

        in_maps.append(dict(
            wqkvT=wqc,
            kT=ktc,
            vaug=vc,
            owT=np.ascontiguousarray(
                np.asarray(o_w, f32)[:, D * c:D * (c + 1)].T.astype(bf16)),
            consts=consts,
        ))
    return in_maps, n_c, s_p


def kernel(**inputs):
    from concourse.bass_utils import run_bass_kernel_spmd

    in_maps, n_c, s_p = _prep_shards(**inputs)
    key = (n_c, s_p)
    if key not in _GRAPH_CACHE:
        _GRAPH_CACHE[key] = _build_graph(n_c, s_p)
    nc = _GRAPH_CACHE[key]

    res = run_bass_kernel_spmd(nc, in_maps, core_ids=list(range(N_CORES)))
    out = np.zeros(HID, np.float64)
    for r in res.results:
        out += r["out"].reshape(HID).astype(np.float64)
    return out.astype(np.float32).reshape(1, HID, 1, 1)


# revision 18
# speedup vs baseline: 1.1746x; 1.0403x over previous
"""Trainium2 Bass kernel for ANE-Gemma MQA single-token decode attention.

Distribution over 8 NeuronCores:
  - QKV projection: output-row sharded (320 rows/core) + AllGather.
  - Attention: KV-cache sequence-sharded; per-core partial softcapped
    attention with fixed exp(s-50) stabilizer; ReduceScatter(add) gives
    core c the summed (acc, l) for head c.
  - O-projection: head-column sharded; per-core 2048-float partials are
    summed on the host.

Host-side prep is layout only: slicing, transposes, replication of tiny
constants, and reading the mask to select valid cache rows (exp(mask) is
folded into the shipped V rows / softmax-denominator column, which is
mathematically identical to the reference's additive mask).
"""

import numpy as np

N_CORES = 8
H = 8            # query heads
D = 256          # head dim
HID = 2048       # hidden
QKV_ROWS = (H + 2) * D          # 2560
ROWS_PER_CORE = QKV_ROWS // N_CORES  # 320
LAYER_INDEX = 5
SOFTCAP = 50.0
OWNER = N_CORES - 1  # core that contributes the freshly-written kv position

_GRAPH_CACHE = {}


def _split_excess_waits(nc):
    """Walrus in this environment accepts at most 1 semaphore wait per
    instruction (2 for EventSemaphore). Tile's wait assigner can emit more;
    hoist the excess into standalone EventSemaphore waits just before the
    instruction on the same engine stream."""
    import concourse.mybir as mybir

    uid = [0]
    for fn in nc.m.functions:
        for blk in fn.blocks:
            out = []
            for inst in blk.instructions:
                si = inst.sync_info
                cap = 2 if isinstance(inst, mybir.InstEventSemaphore) else 1
                if si is not None and si.on_wait and len(si.on_wait) > cap:
                    waits = list(si.on_wait)
                    keep, hoist = waits[-cap:], waits[:-cap]
                    while hoist:
                        chunk, hoist = hoist[:2], hoist[2:]
                        uid[0] += 1
                        out.append(mybir.InstEventSemaphore(
                            name=f"splitw-{uid[0]}",
                            ins=[], outs=[],
                            engine=inst.engine,
                            sync_info=mybir.SyncInfo(on_wait=chunk, on_update=[]),
                        ))
                    inst.sync_info = mybir.SyncInfo(
                        on_wait=keep, on_update=si.on_update)
                out.append(inst)
            if len(out) != len(blk.instructions):
                blk.instructions[:] = out
    return nc


def _build_graph(n_c, s_p, split_waits=True):
    """SPMD Bass graph. n_c real cache rows per core (multiple of 128); the
    new-kv vector occupies row n_c (partition 0 of the last seq tile);
    s_p = n_c + 128."""
    import concourse.bass as bass
    import concourse.mybir as mybir
    from concourse import masks, tile

    fp = mybir.dt.float32
    bf = mybir.dt.bfloat16
    AF = mybir.ActivationFunctionType
    nt = s_p // 128
    assert s_p == n_c + 128 and n_c % 128 == 0

    nc = bass.Bass(num_devices=N_CORES)

    # --- kernel I/O (per-core shards supplied by the host) ---
    # wqkvT carries the hidden-state vector as its last column (321 = 320+1)
    # so each qkv matmul depends on exactly one DMA.
    wq_p = nc.declare_dram_parameter(
        "wqkvT", [HID, ROWS_PER_CORE + 1], bf, isOutput=False)
    kt_p = nc.declare_dram_parameter("kT", [D, s_p], bf, isOutput=False)
    v_p = nc.declare_dram_parameter("vaug", [s_p, D + 1], bf, isOutput=False)
    ow_p = nc.declare_dram_parameter("owT", [D, HID], bf, isOutput=False)
    cst_p = nc.declare_dram_parameter("consts", [36, D], fp, isOutput=False)
    wsel_p = nc.declare_dram_parameter("wsel", [N_CORES * H, 1], fp, isOutput=False)
    out_p = nc.declare_dram_parameter("out", [1, HID], fp, isOutput=True)

    # --- internal DRAM bounce buffers for collectives ---
    cc1_in = nc.dram_tensor("cc1_in", [1, ROWS_PER_CORE], fp)
    cc1_out = nc.dram_tensor("cc1_out", [H + 2, D], fp, addr_space="Shared")
    cc2_in = nc.dram_tensor("cc2_in", [H, D + 1], fp)
    cc2_out = nc.dram_tensor("cc2_out", [N_CORES * H, D + 1], fp,
                             addr_space="Shared")
    rgroups = [list(range(N_CORES))]

    with tile.TileContext(nc) as tc:
        with (
            tc.tile_pool(name="wp", bufs=1) as wp,
            tc.tile_pool(name="sp", bufs=1) as sp,
            tc.tile_pool(name="pp", bufs=8, space="PSUM") as pp,
        ):
            # ---------------- DMA in ----------------
            # critical path first (sync queue): qkv weight slices (+h), consts
            wqv = wq_p.rearrange("(a p) r -> a p r", p=128)  # [16,128,321]
            wq = []
            for a in range(4):
                t = wp.tile([128, 4, ROWS_PER_CORE + 1], bf,
                            name=f"wq{a}", tag=f"wq{a}")
                nc.sync.dma_start(
                    out=t[:],
                    in_=wqv[4 * a:4 * (a + 1)].rearrange("a p r -> p a r"),
                )
                wq.append(t)
            csb = wp.tile([9, 4, D], fp)
            nc.sync.dma_start(
                out=csb[:], in_=cst_p.rearrange("(j r) d -> r j d", r=9))
            cw = csb[:, 0, :]      # norm weights: q rows raw, k row 15+16*kw
            ccos = csb[:, 1, :]
            csin = csb[:, 2, :]
            cfac = csb[0:1, 3, 0:1]  # new-kv mask factor
            # bulk loads on the scalar HWDGE queue: K^T, V, o_w^T
            kt0 = wp.tile([128, s_p], bf)
            kt1 = wp.tile([128, s_p], bf)
            nc.scalar.dma_start(out=kt0[:], in_=kt_p[0:128, :])
            nc.scalar.dma_start(out=kt1[:], in_=kt_p[128:256, :])
            vt = []
            for t_i in range(nt):
                t = wp.tile([128, D + 1], bf, name=f"vt{t_i}", tag=f"vt{t_i}")
                nc.scalar.dma_start(
                    out=t[:], in_=v_p[128 * t_i:128 * (t_i + 1), :]
                )
                vt.append(t)
            ow = []
            for j in range(2):
                for b in range(4):
                    t = wp.tile([128, 512], bf, name=f"ow{j}{b}", tag=f"ow{j}{b}")
                    nc.scalar.dma_start(
                        out=t[:],
                        in_=ow_p[128 * j:128 * (j + 1), 512 * b:512 * (b + 1)],
                    )
                    ow.append(t)

            id16 = wp.tile([16, 16], fp)
            masks.make_identity(nc, id16[:])
            # preload ACT LUTs for tanh/exp during the DMA phase so the
            # real activations later don't pay the ~1.5us table switch
            warm = sp.tile([1, 1], fp)
            nc.gpsimd.memset(warm[:], 0.0)
            nc.scalar.activation(warm[:], warm[:], AF.Tanh)
            nc.scalar.activation(warm[:], warm[:], AF.Exp)

            # ---------------- QKV projection (partial rows) ----------------
            psq = pp.tile([1, ROWS_PER_CORE], fp, tag="ps")
            for k in range(16):
                a, j = k // 4, k % 4
                nc.tensor.matmul(
                    psq[:],
                    lhsT=wq[a][:, j, ROWS_PER_CORE:ROWS_PER_CORE + 1],
                    rhs=wq[a][:, j, 0:ROWS_PER_CORE],
                    start=(k == 0), stop=(k == 15),
                )
            qkvp = sp.tile([1, ROWS_PER_CORE], fp)
            nc.scalar.activation(qkvp[:], psq[:], AF.Copy)
            nc.gpsimd.dma_start(out=cc1_in[:], in_=qkvp[:])

            # ---------------- AllGather qkv ----------------
            nc.gpsimd.collective_compute(
                "AllGather", mybir.AluOpType.bypass, replica_groups=rgroups,
                ins=[cc1_in[:]], outs=[cc1_out[:]],
            )
            qkn = sp.tile([9, D], fp)      # q heads + k
            vrow = sp.tile([1, D], fp)     # raw v
            nc.sync.dma_start(out=qkn[:], in_=cc1_out[0:9, :])
            nc.scalar.dma_start(out=vrow[:], in_=cc1_out[9:10, :])

            # ---------------- RMSNorm + RoPE (q heads + k) ----------------
            # x/||x||*sqrt(D) == ane_rmsnorm's max-prenormalized form in
            # exact arithmetic; f32 cannot overflow at these magnitudes.
            xs2 = sp.tile([9, D], fp)
            nc.vector.tensor_mul(xs2[:], qkn[:], qkn[:])
            ss = sp.tile([9, 1], fp)
            nc.vector.tensor_reduce(
                ss[:], xs2[:], axis=mybir.AxisListType.X, op=mybir.AluOpType.add)
            sq = sp.tile([9, 1], fp)
            nc.scalar.activation(sq[:], ss[:], AF.Sqrt)
            rs = sp.tile([9, 1], fp)
            nc.vector.reciprocal(rs[:], sq[:])
            # q rows: rs*sqrt(D)*SCALING = rs; k row's *16 and the (1+w)
            # offset are baked into cw by the host (cw = 1+w, k: 16*(1+kw)).
            xn = sp.tile([9, D], fp)
            nc.vector.tensor_scalar_mul(xn[:], qkn[:], rs[:])
            xnw = sp.tile([9, D], fp)
            nc.vector.tensor_mul(xnw[:], xn[:], cw[:])
            # rope, exploiting cos/sin half-duplication (emb = [freqs, freqs])
            ca = sp.tile([9, D], fp)
            nc.vector.tensor_mul(ca[:], xnw[:], ccos[:])
            cb = sp.tile([9, D], fp)
            nc.vector.tensor_mul(cb[:], xnw[:], csin[:])
            qr = sp.tile([9, D], fp)
            nc.vector.tensor_sub(qr[:, 0:128], ca[:, 0:128], cb[:, 128:256])
            nc.vector.tensor_add(qr[:, 128:256], ca[:, 128:256], cb[:, 0:128])
            # raw v scaled by the per-core new-kv factor (exp(mask[p]) or 0)
            vscl = sp.tile([1, D], fp)
            nc.vector.tensor_scalar_mul(vscl[:], vrow[:], cfac[:])

            # ---------------- transpose new q/k ----------------
            pst0 = pp.tile([128, 9], fp, tag="ps")
            pst1 = pp.tile([128, 9], fp, tag="ps")
            nc.tensor.transpose(pst0[:], qr[:, 0:128], id16[0:9, 0:9])
            nc.tensor.transpose(pst1[:], qr[:, 128:256], id16[0:9, 0:9])
            qt0 = sp.tile([128, H], bf)
            qt1 = sp.tile([128, H], bf)
            nc.vector.tensor_copy(qt0[:], pst0[:, 0:H])
            nc.vector.tensor_copy(qt1[:], pst1[:, 0:H])
            # append new k as column n_c of K^T
            nc.vector.tensor_copy(kt0[:, n_c:n_c + 1], pst0[:, H:H + 1])
            nc.vector.tensor_copy(kt1[:, n_c:n_c + 1], pst1[:, H:H + 1])
            # append new v as row n_c = partition 0 of the last V tile
            nc.vector.tensor_copy(vt[nt - 1][0:1, 0:D], vscl[:])

            # ---------------- scores + softcap softmax numerators ----------------
            pss = pp.tile([128, nt * H], fp, tag="ps")
            for t_i in range(nt):
                nc.tensor.matmul(
                    pss[:, H * t_i:H * (t_i + 1)],
                    lhsT=kt0[:, 128 * t_i:128 * (t_i + 1)], rhs=qt0[:],
                    start=True, stop=False,
                )
                nc.tensor.matmul(
                    pss[:, H * t_i:H * (t_i + 1)],
                    lhsT=kt1[:, 128 * t_i:128 * (t_i + 1)], rhs=qt1[:],
                    start=False, stop=True,
                )
            nb = sp.tile([128, 1], fp)
            nc.gpsimd.memset(nb[:], -SOFTCAP)
            t40 = sp.tile([128, nt * H], fp)
            nc.scalar.activation(t40[:], pss[:], AF.Tanh, scale=1.0 / SOFTCAP)
            u40 = sp.tile([128, nt * H], bf)
            nc.scalar.activation(u40[:], t40[:], AF.Exp, bias=nb[:], scale=SOFTCAP)

            # ---------------- probs @ [V | 1] ----------------
            psav = pp.tile([H, D + 1], fp, tag="ps")
            for t_i in range(nt):
                nc.tensor.matmul(
                    psav[:], lhsT=u40[:, H * t_i:H * (t_i + 1)], rhs=vt[t_i][:],
                    start=(t_i == 0), stop=(t_i == nt - 1),
                )
            avs = sp.tile([H, D + 1], fp)
            nc.vector.tensor_copy(avs[:], psav[:])
            nc.sync.dma_start(out=cc2_in[:], in_=avs[:])

            # ---------------- AllGather partial (acc, l) ----------------
            nc.gpsimd.collective_compute(
                "AllGather", mybir.AluOpType.bypass, replica_groups=rgroups,
                ins=[cc2_in[:]], outs=[cc2_out[:]],
            )
            wsel = sp.tile([N_CORES * H, 1], fp)
            nc.sync.dma_start(out=wsel[:], in_=wsel_p[:])
            pacc = sp.tile([N_CORES * H, D + 1], fp)
            nc.sync.dma_start(out=pacc[:], in_=cc2_out[:])
            # sum this core's head across ranks: one-hot-weighted reduction
            psacc = pp.tile([1, D + 1], fp, tag="ps")
            nc.tensor.matmul(psacc[:], lhsT=wsel[:], rhs=pacc[:],
                             start=True, stop=True)
            accflat = sp.tile([1, D + 1], fp)
            nc.vector.tensor_copy(accflat[:], psacc[:])
            rl = sp.tile([1, 1], fp)
            nc.vector.reciprocal(rl[:], accflat[0:1, D:D + 1])
            pta = pp.tile([128, 1], fp, tag="ps")
            ptb = pp.tile([128, 1], fp, tag="ps")
            nc.tensor.transpose(pta[:], accflat[0:1, 0:128], id16[0:1, 0:1])
            nc.tensor.transpose(ptb[:], accflat[0:1, 128:256], id16[0:1, 0:1])
            acc2 = sp.tile([128, 2], bf)
            nc.vector.tensor_copy(acc2[:, 0:1], pta[:])
            nc.vector.tensor_copy(acc2[:, 1:2], ptb[:])

            # ---------------- O-projection partial ----------------
            osb = sp.tile([1, HID], fp)
            for b in range(4):
                pso = pp.tile([1, 512], fp, name=f"pso{b}", tag="ps")
                nc.tensor.matmul(pso[:], lhsT=acc2[:, 0:1], rhs=ow[b][:],
                                 start=True, stop=False)
                nc.tensor.matmul(pso[:], lhsT=acc2[:, 1:2], rhs=ow[4 + b][:],
                                 start=False, stop=True)
                if b % 2 == 0:
                    nc.vector.tensor_scalar_mul(
                        osb[0:1, 512 * b:512 * (b + 1)], pso[:], rl[:])
                else:
                    nc.scalar.activation(
                        osb[0:1, 512 * b:512 * (b + 1)], pso[:], AF.Copy,
                        scale=rl[:])
            nc.sync.dma_start(out=out_p[:], in_=osb[:])

    return _split_excess_waits(nc) if split_waits else nc


def _prep_shards(hidden_states, cos, sin, kv_write_indices, k_cache, v_cache,
                 mask, qkv_w, o_w, q_norm_w, k_norm_w):
    import ml_dtypes
    f32 = np.float32
    bf16 = ml_dtypes.bfloat16
    p = int(np.asarray(kv_write_indices))
    mask_flat = np.asarray(mask, f32).reshape(-1)
    seq = mask_flat.shape[0]

    valid = np.nonzero(mask_flat > -1e8)[0]
    rows = valid[valid != p]
    n_c = max(1, (len(rows) + N_CORES - 1) // N_CORES)
    n_c = ((n_c + 127) // 128) * 128   # new-kv row lands at partition 0
    s_p = n_c + 128

    idx = np.zeros(N_CORES * n_c, np.int64)
    idx[:len(rows)] = rows
    live = np.zeros(N_CORES * n_c, bool)
    live[:len(rows)] = True
    idx = idx.reshape(N_CORES, n_c)
    live = live.reshape(N_CORES, n_c)

    k_l = np.asarray(k_cache, f32)[LAYER_INDEX, 0]
    v_l = np.asarray(v_cache, f32)[LAYER_INDEX, 0]

    h_vec = np.asarray(hidden_states, f32).reshape(HID)
    wqT = np.asarray(qkv_w, f32).T  # [HID, 2560]
    cos_f = np.asarray(cos, f32).reshape(D)
    sin_f = np.asarray(sin, f32).reshape(D)
    qw = np.asarray(q_norm_w, f32).reshape(D)
    kw = np.asarray(k_norm_w, f32).reshape(D)

    in_maps = []
    for c in range(N_CORES):
        rows_c = idx[c]
        live_c = live[c]
        # mask factor per shipped row: exp(mask) for live rows, 0 for padding
        mfac = np.zeros(n_c, f32)
        mfac[live_c] = np.exp(
            mask_flat[rows_c[live_c]].astype(np.float64)).astype(f32)

        ktc = np.zeros((D, s_p), bf16)
        ktc[:, :n_c] = k_l[rows_c].T.astype(bf16)
        vc = np.zeros((s_p, D + 1), bf16)
        vc[:n_c, :D] = (v_l[rows_c] * mfac[:, None]).astype(bf16)
        vc[:n_c, D] = mfac.astype(bf16)
        # new-kv slot at row n_c: factor = exp(mask[p]) on the owner core only
        nf = f32(0.0)
        if c == OWNER and 0 <= p < seq:
            nf = np.exp(np.float64(mask_flat[p])).astype(f32)
        vc[n_c, D] = bf16(nf)

        consts = np.zeros((36, D), f32)
        consts[0:8] = 1.0 + qw
        consts[8] = 16.0 + 16.0 * kw   # 16*(1+kw): folds in sqrt(D)
        consts[9:18] = cos_f
        consts[18:27] = sin_f
        consts[27, 0] = nf

        wqc = np.zeros((HID, ROWS_PER_CORE + 1), bf16)
        wqc[:, :ROWS_PER_CORE] = wqT[
            :, ROWS_PER_CORE * c:ROWS_PER_CORE * (c + 1)].astype(bf16)
        wqc[:, ROWS_PER_CORE] = h_vec.astype(bf16)

        wsel = np.zeros((N_CORES * H, 1), f32)
        wsel[np.arange(N_CORES) * H + c, 0] = 1.0

        in_maps.append(dict(
            wsel=wsel,
            wqkvT=wqc,
            kT=ktc,
            vaug=vc,
            owT=np.ascontiguousarray(
                np.asarray(o_w, f32)[:, D * c:D * (c + 1)].T.astype(bf16)),
            consts=consts,
        ))
    return in_maps, n_c, s_p


def kernel(**inputs):
    from concourse.bass_utils import run_bass_kernel_spmd

    in_maps, n_c, s_p = _prep_shards(**inputs)
    key = (n_c, s_p)
    if key not in _GRAPH_CACHE:
        _GRAPH_CACHE[key] = _build_graph(n_c, s_p)
    nc = _GRAPH_CACHE[key]

    res = run_bass_kernel_spmd(nc, in_maps, core_ids=list(range(N_CORES)))
    out = np.zeros(HID, np.float64)
    for r in res.results:
        out += r["out"].reshape(HID).astype(np.float64)
    return out.astype(np.float32).reshape(1, HID, 1, 1)
